# revision 1
# baseline (speedup 1.0000x reference)
"""Trainium2 Bass kernel for nn_AdaptiveFullConnected (segment_reduce).

Reference computation (per batch b):
    c      = coords + depthwise_conv1d(coords, K=5) + conv_b          [N, 2]
    h      = gelu(c @ lin1_w.T + lin1_b)                              [N, 512]
    weight = h @ lin2_w.T + lin2_b                                    [N, 512]
    xw     = tile(x, 8) * weight                                      [N, 512]
    mean_p = mean over {n : idx[n] == p} of xw[n, :]                  [P, 512]
    out    = w1 * sin(mean) + w2 * cos(mean)                          [P, 512]

Sharding: 8 cores = (batch b = core//2) x (half of N = core%2), 8192 rows
per core.  Each core computes partial segment sums for all 256 segments as
a one-hot matmul, a pairwise ReduceScatter combines the two halves (core
2b keeps segments 0:128, core 2b+1 keeps 128:256), and the epilogue
(bias-fold, mean, sin/cos) runs on the 128 rows each core owns.

The lin2 bias is folded through the segment reduce:
    seg(x * (w_nb + b2)) = seg(x * w_nb) + b2 * seg(x)
so the device never materializes a bias add over [N, 512]; instead the
segment matmul carries 577 columns: 512 for x*w_nb, 64 for seg(x) (x has
only 64 unique columns), 1 for the segment counts.
"""

import numpy as np
from contextlib import ExitStack

B = 4
N = 16384
DIMS = 64
HEADS = 8
D = DIMS * HEADS  # 512
K = 5
PFULL = 256
NCORES = 8
NLOC = N // 2  # 8192 rows per core
NT = NLOC // 128  # 64 k-tiles
CHUNK = 512  # n-chunk for lin1/lin2
NCH = NLOC // CHUNK  # 16
ET = D // 128  # 4 e-tiles
SEGW = D + DIMS + 1  # 577
GROUPS = [[0, 1], [2, 3], [4, 5], [6, 7]]

_CACHE = {}


def build_nc():
    import concourse.bass as bass  # noqa: F401
    import concourse.mybir as mybir
    import concourse.tile as tile
    from concourse import bacc

    f16 = mybir.dt.float16
    f32 = mybir.dt.float32
    f8 = mybir.dt.float8e4
    i32 = mybir.dt.int32
    DR = mybir.MatmulPerfMode.DoubleRow
    mult = mybir.AluOpType.mult
    add = mybir.AluOpType.add
    is_equal = mybir.AluOpType.is_equal
    AF = mybir.ActivationFunctionType

    nc = bacc.Bacc("TRN2", num_devices=NCORES)

    x16 = nc.declare_dram_parameter("x16", [128, NT * DIMS], f16, isOutput=False)
    idxs = nc.declare_dram_parameter("idxs", [128, NT], i32, isOutput=False)
    conv_in = nc.declare_dram_parameter("conv_in", [128, 132], f32, isOutput=False)
    w1aug = nc.declare_dram_parameter("w1aug", [2, D], f16, isOutput=False)
    w2t = nc.declare_dram_parameter("w2t", [128, ET * D], f16, isOutput=False)
    b2rep = nc.declare_dram_parameter("b2rep", [128, D], f32, isOutput=False)
    consts = nc.declare_dram_parameter("consts", [128, 16], f32, isOutput=False)
    out = nc.declare_dram_parameter("out", [128, D], f32, isOutput=True)

    with tile.TileContext(nc, num_cores=NCORES) as tc, ExitStack() as ctx:
        cpool = ctx.enter_context(tc.tile_pool(name="cpool", bufs=1))
        work = ctx.enter_context(tc.tile_pool(name="work", bufs=1))
        psum = ctx.enter_context(tc.tile_pool(name="psum", bufs=1, space="PSUM"))
        dram = ctx.enter_context(tc.tile_pool(name="dram", bufs=1, space="DRAM"))

        # ---- constant loads (conv-critical path first: it gates the first matmul) ----
        cT = cpool.tile([2, NLOC], f16)
        ci_sb = cpool.tile([128, 132], f32)
        nc.sync.dma_start(out=ci_sb[:], in_=conv_in[:])
        cst = cpool.tile([128, 16], f32)
        nc.sync.dma_start(out=cst[:], in_=consts[:])
        w1_sb = cpool.tile([2, D], f16)
        nc.sync.dma_start(out=w1_sb[:], in_=w1aug[:])
        idx_sb = cpool.tile([128, NT], i32)
        nc.sync.dma_start(out=idx_sb[:], in_=idxs[:])
        iota_sb = cpool.tile([128, PFULL], i32)
        nc.gpsimd.iota(iota_sb[:], pattern=[[1, PFULL]], base=0, channel_multiplier=0)

        # PE warm-up: the HAM clock gate keeps PE at 1.2 GHz until ~3.4us of
        # sustained activity; burn junk matmuls on a zeroed tile while the
        # conv chain runs so the real matmuls start at 2.4 GHz
        zt = cpool.tile([128, 256], f16)
        nc.gpsimd.memset(zt[:], 0.0)
        pwarm = psum.tile([128, 256], f32, name="pwarm", tag="ph", bufs=2)
        for _ in range(90):
            nc.tensor.matmul(
                pwarm[:], lhsT=zt[:, 0:128], rhs=zt[:], start=True, stop=True
            )

        # ---- depthwise conv on coords ----
        # ci_sb row p = (ch, r): ci_sb[p, j] = coords_pad[r*128 + j, ch]
        # local n = r*128 + jj :  c[n] = ci[jj+2] + conv_b + sum_k w_k * ci[jj+k]
        with tc.high_priority():
            acc0 = work.tile([128, 128], f32, name="acc0")
            acc1 = work.tile([128, 128], f32, name="acc1")
            nc.vector.tensor_scalar(
                out=acc0[:], in0=ci_sb[:, 0:128], scalar1=cst[:, 0:1], scalar2=None, op0=mult
            )
            accs = [acc0, acc1]
            for k in range(1, K):
                src, dst = accs[(k + 1) % 2], accs[k % 2]
                nc.vector.scalar_tensor_tensor(
                    out=dst[:], in0=ci_sb[:, k : k + 128], scalar=cst[:, k : k + 1],
                    in1=src[:], op0=mult, op1=add,
                )
            # after k=4 the live accumulator is accs[0]
            cfin = work.tile([128, 128], f16, name="cfin")
            nc.vector.scalar_tensor_tensor(
                out=cfin[:], in0=ci_sb[:, 2:130], scalar=cst[:, 5:6], in1=accs[0][:],
                op0=add, op1=add,
            )
            # shuffle [128=(ch,r), 128j] -> [2, (r j)] directly SBUF->SBUF,
            # first-chunk split so chunk 0's lin1 starts early
            ct_dmas = []
            for ch in range(2):
                ct_dmas.append(nc.sync.dma_start(
                    out=cT[ch : ch + 1, 0:CHUNK],
                    in_=cfin[ch * 64 : ch * 64 + 4, :],
                ))
                nc.sync.dma_start(
                    out=cT[ch : ch + 1, CHUNK:NLOC],
                    in_=cfin[ch * 64 + 4 : (ch + 1) * 64, :],
                )

        # bulk loads, explicitly ordered after the cT first-chunk readback so
        # their HBM traffic can't delay the conv-critical chain's completion
        from concourse.bass import _add_dep_helper

        x_sb = cpool.tile([128, NT, DIMS], f16)
        x_dma = nc.scalar.dma_start(
            out=x_sb[:], in_=x16[:].rearrange("p (t c) -> p t c", c=DIMS)
        )
        w2_sb = cpool.tile([128, ET, D], f16)
        w2_dma = nc.gpsimd.dma_start(
            out=w2_sb[:], in_=w2t[:].rearrange("p (e d) -> p e d", d=D)
        )
        for bulk in (x_dma, w2_dma):
            for ctd in ct_dmas:
                _add_dep_helper(bulk.ins, ctd.ins, sync=True,
                                reason="bulk load after conv-critical cT readback")

        # ---- persistent segment accumulators (PSUM, 2 banks each) ----
        pseg = [psum.tile([128, SEGW], f32, name=f"pseg{i}") for i in range(2)]
        # two-phase k-split (in units of DoubleRow k-PAIRS): phase A's
        # reduce-scatter overlaps phase B's compute
        NPAIR = NT // 2
        PSPLIT = NPAIR // 2
        seg_parts = [dram.tile([PFULL, SEGW], f16, name=f"seg_part{i}") for i in range(2)]
        seg_reds = [dram.tile([128, SEGW], f16, name=f"seg_red{i}") for i in range(2)]

        def drain_and_reduce(phase):
            for p2 in range(2):
                s = work.tile([128, SEGW], f16, name=f"seg_sb{phase}{p2}")
                if p2 == 0:
                    nc.vector.tensor_copy(out=s[:], in_=pseg[p2][:])
                else:
                    nc.scalar.copy(out=s[:], in_=pseg[p2][:])
                nc.sync.dma_start(
                    out=seg_parts[phase][p2 * 128 : (p2 + 1) * 128, :], in_=s[:]
                )
            nc.gpsimd.collective_compute(
                "ReduceScatter",
                mybir.AluOpType.add,
                replica_groups=GROUPS,
                ins=[seg_parts[phase][:]],
                outs=[seg_reds[phase][:]],
            )

        # ---- main loop ----
        for c in range(NCH):
            hts = []
            for e in range(ET):
                ph = psum.tile([128, CHUNK], f32, name="ph", bufs=2)
                nc.tensor.matmul(
                    ph[:],
                    lhsT=w1_sb[:, e * 128 : (e + 1) * 128],
                    rhs=cT[:, c * CHUNK : (c + 1) * CHUNK],
                    start=True, stop=True,
                )
                ht = work.tile([128, CHUNK], f16, name=f"ht{e}", bufs=2)
                nc.scalar.activation(
                    out=ht[:], in_=ph[:], func=AF.Gelu, bias=cst[:, 9 + e : 10 + e]
                )
                hts.append(ht)
            for tp in range(2):
                pair = c * 2 + tp
                # DoubleRow fp8 pair tiles: [:, h, :] holds ntile 2*pair+h
                xwp = work.tile([128, 2, SEGW], f8, name="xwp", bufs=3)
                ohp = work.tile([128, 2, PFULL], f8, name="ohp", bufs=3)
                for h in range(2):
                    t4 = tp * 2 + h
                    kt = c * 4 + t4
                    pw = psum.tile([128, D], f32, name="pw", bufs=2)
                    for e in range(ET):
                        nc.tensor.matmul(
                            pw[:],
                            lhsT=hts[e][:, t4 * 128 : (t4 + 1) * 128],
                            rhs=w2_sb[:, e, :],
                            start=(e == 0), stop=(e == ET - 1),
                        )
                    xv = x_sb[:, kt, :].unsqueeze(1).to_broadcast([128, HEADS, DIMS])
                    nc.vector.tensor_tensor(
                        out=xwp[:, h, 0:D].rearrange("p (hh c) -> p hh c", c=DIMS),
                        in0=pw[:].rearrange("p (hh c) -> p hh c", c=DIMS),
                        in1=xv, op=mult,
                    )
                    nc.vector.tensor_copy(
                        out=xwp[:, h, D : D + DIMS], in_=x_sb[:, kt, :]
                    )
                    nc.gpsimd.memset(xwp[:, h, D + DIMS : SEGW], 1.0)
                    nc.vector.tensor_tensor(
                        out=ohp[:, h, :],
                        in0=idx_sb[:, kt : kt + 1].to_broadcast([128, PFULL]),
                        in1=iota_sb[:], op=is_equal,
                    )
                ph_start = pair == 0 or pair == PSPLIT
                ph_stop = pair == PSPLIT - 1 or pair == NPAIR - 1
                for p2 in range(2):
                    lhs = ohp[:, :, p2 * 128 : (p2 + 1) * 128]
                    nc.tensor.matmul(
                        pseg[p2][:, 0:D], lhsT=lhs, rhs=xwp[:, :, 0:D],
                        start=ph_start, stop=ph_stop, perf_mode=DR,
                    )
                    nc.tensor.matmul(
                        pseg[p2][:, D:SEGW], lhsT=lhs, rhs=xwp[:, :, D:SEGW],
                        start=ph_start, stop=ph_stop, perf_mode=DR,
                    )
                if pair == PSPLIT - 1:
                    drain_and_reduce(0)

        redA = work.tile([128, SEGW], f16, name="redA")
        nc.sync.dma_start(out=redA[:], in_=seg_reds[0][:])
        drain_and_reduce(1)

        # ---- epilogue on the 128 owned segments ----
        b2_sb = cpool.tile([128, D], f32)
        nc.gpsimd.dma_start(out=b2_sb[:], in_=b2rep[:])
        redB = work.tile([128, SEGW], f16, name="redB")
        nc.sync.dma_start(out=redB[:], in_=seg_reds[1][:])
        red = work.tile([128, SEGW], f32, name="red")
        nc.vector.tensor_tensor(out=red[:], in0=redA[:], in1=redB[:], op=add)
        rec = work.tile([128, 1], f32, name="rec")
        nc.vector.reciprocal(out=rec[:], in_=red[:, SEGW - 1 : SEGW])
        t1 = work.tile([128, D], f32, name="t1")
        xsegv = red[:, D : D + DIMS].unsqueeze(1).to_broadcast([128, HEADS, DIMS])
        nc.vector.tensor_tensor(
            out=t1[:].rearrange("p (h c) -> p h c", c=DIMS),
            in0=b2_sb[:].rearrange("p (h c) -> p h c", c=DIMS),
            in1=xsegv, op=mult,
        )
        t2 = work.tile([128, D], f32, name="t2")
        nc.vector.tensor_tensor(out=t2[:], in0=t1[:], in1=red[:, 0:D], op=add)
        sinp = work.tile([128, D], f32, name="sinp")
        nc.scalar.activation(out=sinp[:], in_=t2[:], func=AF.Sin, scale=rec[:, 0:1])
        cosp = work.tile([128, D], f32, name="cosp")
        nc.scalar.activation(
            out=cosp[:], in_=t2[:], func=AF.Sin, bias=cst[:, 6:7], scale=rec[:, 0:1]
        )
        sins = work.tile([128, D], f32, name="sins")
        nc.vector.tensor_scalar(
            out=sins[:], in0=sinp[:], scalar1=cst[:, 7:8], scalar2=None, op0=mult
        )
        out_sb = work.tile([128, D], f32, name="out_sb")
        nc.vector.scalar_tensor_tensor(
            out=out_sb[:], in0=cosp[:], scalar=cst[:, 8:9], in1=sins[:],
            op0=mult, op1=add,
        )
        nc.sync.dma_start(out=out[:], in_=out_sb[:])

    nc.finalize()
    return nc


def make_in_maps(x, coords, indices, conv_w, conv_b, lin1_w, lin1_b, lin2_w,
                 lin2_b, w1, w2):
    """Host-side sharding + layout prep.  Returns list of 8 input dicts."""
    x = np.asarray(x, np.float32)
    coords = np.asarray(coords, np.float32)
    idx_full = np.asarray(indices).reshape(B, N).astype(np.int32)
    conv_w = np.asarray(conv_w, np.float32)
    conv_b = np.asarray(conv_b, np.float32)
    lin1_w = np.asarray(lin1_w, np.float32)
    lin1_b = np.asarray(lin1_b, np.float32)
    lin2_w = np.asarray(lin2_w, np.float32)
    lin2_b = np.asarray(lin2_b, np.float32)

    w1aug = np.stack([lin1_w[:, 0], lin1_w[:, 1]]).astype(np.float16)  # [2, D]
    w2t = np.ascontiguousarray(lin2_w.T)  # [e, d]
    w2t_sh = (
        w2t.reshape(ET, 128, D).transpose(1, 0, 2).reshape(128, ET * D)
        .astype(np.float16)
    )
    b2rep = np.tile(lin2_b[None, :], (128, 1)).astype(np.float32)
    consts = np.zeros((128, 16), np.float32)
    ch_of_p = (np.arange(128) // 64)  # 0 for rows 0:64, 1 for 64:128
    for k in range(K):
        consts[:, k] = conv_w[ch_of_p, 0, k]
    consts[:, 5] = conv_b[ch_of_p]
    consts[:, 6] = np.pi / 2
    consts[:, 9:13] = lin1_b.reshape(4, 128).T
    consts[:, 7] = np.float32(np.asarray(w1).reshape(-1)[0])
    consts[:, 8] = np.float32(np.asarray(w2).reshape(-1)[0])

    r = np.arange(64)
    j = np.arange(132)
    halo_idx = r[:, None] * 128 + j[None, :]  # [64, 132] indices into coords_pad

    in_maps = []
    for core in range(NCORES):
        b, half = core // 2, core % 2
        lo = half * NLOC
        xs = x[b, lo : lo + NLOC, :]  # [8192, 64]
        x_sh = (
            xs.reshape(NT, 128, DIMS).transpose(1, 0, 2).reshape(128, NT * DIMS)
            .astype(np.float16)
        )
        idx_sh = np.ascontiguousarray(
            idx_full[b, lo : lo + NLOC].reshape(NT, 128).T
        ).astype(np.int32)
        cpad = np.zeros((NLOC + 4, 2), np.float32)
        glo, ghi = lo - 2, lo + NLOC + 2
        slo, shi = max(glo, 0), min(ghi, N)
        cpad[slo - glo : shi - glo] = coords[b, slo:shi, :]
        conv_in = np.concatenate(
            [cpad[halo_idx, 0], cpad[halo_idx, 1]], axis=0
        ).astype(np.float32)  # [128, 132]
        in_maps.append(
            dict(
                x16=x_sh, idxs=idx_sh, conv_in=conv_in, w1aug=w1aug, w2t=w2t_sh,
                b2rep=b2rep, consts=consts,
            )
        )
    return in_maps


def assemble(results):
    """[8 x {'out': [128, 512]}] -> [B, PFULL, D] float32."""
    out = np.empty((B, PFULL, D), np.float32)
    for core in range(NCORES):
        b, half = core // 2, core % 2
        out[b, half * 128 : (half + 1) * 128, :] = results[core]["out"]
    return out


def kernel(x, coords, indices, patch_seq_len, conv_w, conv_b, lin1_w, lin1_b,
           lin2_w, lin2_b, w1, w2):
    from concourse.bass_utils import run_bass_kernel_spmd

    if "nc" not in _CACHE:
        _CACHE["nc"] = build_nc()
    nc = _CACHE["nc"]
    in_maps = make_in_maps(x, coords, indices, conv_w, conv_b, lin1_w, lin1_b,
                           lin2_w, lin2_b, w1, w2)
    res = run_bass_kernel_spmd(nc, in_maps, core_ids=list(range(NCORES)))
    return assemble(res.results)

